# revision 3
# baseline (speedup 1.0000x reference)
"""Trainium2 Bass kernel for nn_Conv2d_72052371357971.

Text-CNN style conv stack: three conv groups (k=1,2,3) over [N,3,256]
windows + per-group max-pool, concatenated to [N,256].

Strategy (pure data parallel across 8 NeuronCores):
  * All three conv groups fold into ONE [768, 406] weight matrix over the
    flattened window (3*256 channels). Group outputs needing max-pooling
    occupy disjoint column ranges; pooling is an elementwise max of column
    slices afterwards.
  * Host repacks x into channel-major [128, batch] tiles (free: only device
    time counts) so the contraction dim sits on SBUF partitions.
  * Device, per 128-row batch tile: 7 accumulating matmuls into one PSUM
    bank (6 K-subtiles of 128 + a K=1 ones-row matmul that adds the bias),
    ScalarE copies PSUM->SBUF, VectorE does the pools, DMA streams out
    [batch, 256] rows.
  * DMA is batched into 1024-row super-tiles (1.5 MB loads / 1 MB stores).
"""

import numpy as np

import concourse.bacc as bacc
import concourse.mybir as mybir
import concourse.tile as tile
from concourse.bass import ds
from concourse.bass_utils import run_bass_kernel_spmd

# Problem shapes (hardcoded per contract)
N = 65536
NCORES = 8
B = N // NCORES           # 8192 batch rows per core
TB = 128                  # batch tile (PSUM partition dim)
TPS = 8                   # batch tiles per super-tile
SUP = B // (TPS * TB)     # 8 super-tiles per core
K = 768                   # contraction: 3 positions x 256 channels
KS = K // 128             # 6 K-subtiles
F = 406                   # pre-pool filters: 3*50 + 2*50 + 156
FO = 256                  # output filters after pooling

_F32 = mybir.dt.float32
_cache = {}


def _build_nc(reps=1):
    nc = bacc.Bacc("TRN2", target_bir_lowering=False, debug=False)

    x_d = nc.dram_tensor("x", [SUP, 128, TPS * KS * TB], _F32, kind="ExternalInput")
    w_d = nc.dram_tensor("w", [128, KS * F], _F32, kind="ExternalInput")
    b_d = nc.dram_tensor("b", [1, F], _F32, kind="ExternalInput")
    o_d = nc.dram_tensor("o", [SUP, TPS, TB, FO], _F32, kind="ExternalOutput")

    with tile.TileContext(nc) as tc:
        with (
            tc.tile_pool(name="const", bufs=1) as constp,
            tc.tile_pool(name="xp", bufs=2) as xp,
            tc.tile_pool(name="yp", bufs=4) as yp,
            tc.tile_pool(name="op", bufs=2) as op,
            tc.tile_pool(name="ps", bufs=8, space="PSUM") as psp,
        ):
            wt = constp.tile([128, KS * F], _F32)
            nc.sync.dma_start(wt[:], w_d[:])
            brow = constp.tile([1, F], _F32)
            nc.sync.dma_start(brow[:], b_d[:])
            ones = constp.tile([1, TB], _F32)
            nc.vector.memset(ones[:], 1.0)

            for s in [si for _ in range(reps) for si in range(SUP)]:
                xt = xp.tile([128, TPS * KS * TB], _F32)
                nc.sync.dma_start(xt[:], x_d[s])
                ot = op.tile([128, TPS * FO], _F32)
                for t in range(TPS):
                    acc = psp.tile([128, F], _F32)
                    for j in range(KS):
                        nc.tensor.matmul(
                            acc[:],
                            lhsT=xt[:, ds(t * KS * TB + j * TB, TB)],
                            rhs=wt[:, ds(j * F, F)],
                            start=(j == 0),
                            stop=False,
                        )
                    nc.tensor.matmul(
                        acc[:], lhsT=ones[:], rhs=brow[:], start=False, stop=True
                    )
                    y = yp.tile([128, F], _F32)
                    nc.scalar.activation(
                        y[:], acc[:], mybir.ActivationFunctionType.Copy
                    )
                    o0 = t * FO
                    nc.vector.tensor_max(
                        ot[:, ds(o0, 50)], y[:, ds(0, 50)], y[:, ds(50, 50)]
                    )
                    nc.vector.tensor_max(
                        ot[:, ds(o0, 50)], ot[:, ds(o0, 50)], y[:, ds(100, 50)]
                    )
                    nc.vector.tensor_max(
                        ot[:, ds(o0 + 50, 50)], y[:, ds(150, 50)], y[:, ds(200, 50)]
                    )
                    nc.vector.tensor_copy(
                        ot[:, ds(o0 + 100, 156)], y[:, ds(250, 156)]
                    )
                # SBUF [p, (t f)] -> DRAM [t, p, f]
                nc.sync.dma_start(
                    o_d[s].rearrange("t p f -> p t f"),
                    ot[:].rearrange("p (t f) -> p t f", t=TPS),
                )
    nc.compile()
    return nc


def _pack_weights(W1, b1, W2, b2, W3, b3):
    Wc = np.zeros((K, F), np.float32)
    Wc[0:256, 0:50] = W1.T
    Wc[256:512, 50:100] = W1.T
    Wc[512:768, 100:150] = W1.T
    Wc[0:256, 150:200] = W2[:, 0, :].T
    Wc[256:512, 150:200] = W2[:, 1, :].T
    Wc[256:512, 200:250] = W2[:, 0, :].T
    Wc[512:768, 200:250] = W2[:, 1, :].T
    Wc[:, 250:406] = W3.reshape(156, K).T
    wt = np.ascontiguousarray(
        Wc.reshape(KS, 128, F).transpose(1, 0, 2).reshape(128, KS * F)
    )
    brow = np.concatenate(
        [b1[:, 0], b1[:, 1], b1[:, 2], b2[:, 0], b2[:, 1], b3]
    ).astype(np.float32)[None, :]
    return wt, brow


def kernel(x, W1, b1, W2, b2, W3, b3):
    x = np.ascontiguousarray(x, np.float32)
    wt, brow = _pack_weights(
        np.asarray(W1, np.float32),
        np.asarray(b1, np.float32),
        np.asarray(W2, np.float32),
        np.asarray(b2, np.float32),
        np.asarray(W3, np.float32),
        np.asarray(b3, np.float32),
    )

    if "nc" not in _cache:
        _cache["nc"] = _build_nc()
    nc = _cache["nc"]

    xs = x.reshape(N, K)
    in_maps = []
    for c in range(NCORES):
        xc = xs[c * B : (c + 1) * B]
        # [s, t, f, j, p] -> [s, p, t, j, f] so each super-tile is one
        # contiguous [128, TPS*KS*TB] channel-major block
        arr = np.ascontiguousarray(
            xc.reshape(SUP, TPS, TB, KS, 128).transpose(0, 4, 1, 3, 2)
        ).reshape(SUP, 128, TPS * KS * TB)
        in_maps.append({"x": arr, "w": wt, "b": brow})

    res = run_bass_kernel_spmd(nc, in_maps, list(range(NCORES)))

    outs = []
    for c in range(NCORES):
        o = res.results[c]["o"]  # [SUP, TPS, TB, FO]; (s,t,p) == batch order
        outs.append(np.asarray(o).reshape(B, FO))
    out = np.concatenate(outs, axis=0)
    return out[:, :, None, None]
